# revision 12
# baseline (speedup 1.0000x reference)
"""Minibatch discrimination kernel for 8 trn2 NeuronCores.

reference:
    M = (x @ T).reshape(B, K, D)                       # B=1024, K=50, D=5
    abs_diffs[i,k,j] = sum_d |M[i,k,d] - M[j,k,d]|
    feat[i,k] = sum_j exp(-abs_diffs[i,k,j])
    out = concat([x, feat], axis=1)                    # [1024, 562]

Sharding (symmetric-banded): G[i,j,k] = exp(-abs_diffs) is symmetric in
(i,j).  Core c owns query rows [128c, 128c+128) and computes only the
key band [128c, 128c+512) mod B (its diagonal block + the next 3
128-blocks) -> unordered block pairs at distance 1..3 covered exactly
once; the 4 distance-4 pairs are added by a host-side numpy correction.
Every core's band contributes:
  - row sums over its band  -> feat for its own 128 rows (Exp accum_out)
  - column sums over the 3 off-diagonal chunks (PE matmul with the exp
    tile as stationary operand, ones as moving) -> feat contributions
    for rows owned by cores c+1..c+3, added host-side.

Per-core inner loop (i = 128 local rows on partitions, j = 640 band keys
on the free axis):
 - PE broadcasts row c of M^T via one-hot matmul.  T's columns are
   permuted host-side so consecutive planes land in different 32-row PE
   groups (quadrant overlap).
 - Planes d=0,2: ScalarE Abs(-psum + bias), bias = M_local[:, c].
 - Planes d=1,3,4: custom DVE op  L = |psum - M_local[:,c]| + L_prev
   (ABS_DIFF_ACC) -- abs + plane-accumulation fused in one op.
 - GpSimd adds the second scalar plane into the chain.
 - ScalarE Exp(-L1) with accum_out -> row sums.
"""

import sys

sys.path.insert(0, "/opt/trn_rl_repo")

from contextlib import ExitStack

import numpy as np

import concourse.bass as bass
import concourse.bacc as bacc
import concourse.tile as tile
from concourse import mybir
from concourse.bass_utils import run_bass_kernel_spmd

B, F = 1024, 512
K, D = 50, 5
C = K * D  # 250 columns of M
CPAD = 256  # padded (permuted) column count: 2 tiles of 128 slots
NCORES = 8
ROWS = B // NCORES  # 128 query rows per core
W = 512  # key band width per core (diag block + 3 neighbours)
NCHUNK = 3  # off-diagonal 128-chunks per band

f32 = mybir.dt.float32
f16 = mybir.dt.float16


# ---- custom DVE op: out = |in0 - s0| + in1 ------------------------------
def _ensure_absacc():
    import concourse.dve_ops as dve_ops
    from concourse.dve_spec import C0, Spec, Src0, Src1, maxx

    for op in dve_ops.OPS:
        if op.name == "ABS_DIFF_ACC":
            return op

    def _ref(in0, in1, s0, s1, imm2):
        return (np.abs(in0.astype(np.float32) - s0) + in1).astype(np.float32)

    op = dve_ops.DveOp(
        "ABS_DIFF_ACC",
        Spec(body=maxx(Src0 - C0, C0 - Src0) + Src1, reference=_ref),
        subdim=False,
        uops_sha={"v3": "25e7d27e1dcdc09f", "v4": "1ccaf69ab942959b"},
    )
    dve_ops.OPS.append(op)  # in place: bass_utils holds a from-import binding
    dve_ops._SUB_OPCODE_FOR_NAME[op.name] = (
        dve_ops._CUSTOM_DVE_ROW_BASE + len(dve_ops.OPS) - 1
    )
    return op


ABS_DIFF_ACC = _ensure_absacc()


def _slot_assignment():
    """Map plane c -> slot s so that group(s) = (s%128)//32 == c%4."""
    slot_of = [0] * C
    nxt = {q: 0 for q in range(4)}
    for c in range(C):
        q = c % 4
        i = nxt[q]
        nxt[q] += 1
        blk, r = divmod(i, 32)
        slot_of[c] = 128 * blk + 32 * q + r
    return slot_of


SLOT_OF = _slot_assignment()


def _build_program():
    nc = bacc.Bacc("TRN2", target_bir_lowering=False)

    # per-core rotated inputs: xT columns [0, W) are this core's key band,
    # columns [0, ROWS) are its own query rows
    xT = nc.dram_tensor("xT", [F, W], f32, kind="ExternalInput").ap()
    xTloc = nc.dram_tensor("xTloc", [F, ROWS], f32, kind="ExternalInput").ap()
    Tm = nc.dram_tensor("Tm", [F, CPAD], f32, kind="ExternalInput").ap()
    onehot = nc.dram_tensor("onehot", [128, 32 * 128], f16, kind="ExternalInput").ap()
    feat = nc.dram_tensor("feat", [ROWS, K], f32, kind="ExternalOutput").ap()
    csum = nc.dram_tensor("csum", [128, NCHUNK * K], f32, kind="ExternalOutput").ap()

    with tile.TileContext(nc) as tc, ExitStack() as ctx:
        const_pool = ctx.enter_context(tc.tile_pool(name="const", bufs=1))
        build_ctx = ExitStack()
        build_psum = build_ctx.enter_context(
            tc.tile_pool(name="bpsum", bufs=1, space="PSUM")
        )
        a_pool = ctx.enter_context(tc.tile_pool(name="apool", bufs=6))
        l_pool = ctx.enter_context(tc.tile_pool(name="lpool", bufs=8))
        g_pool = ctx.enter_context(tc.tile_pool(name="gpool", bufs=4))
        scratch_pool = ctx.enter_context(tc.tile_pool(name="scratch", bufs=6))

        # ---- load inputs -------------------------------------------------
        xt_sb = []
        t_sb = []
        xtl_sb = []
        for fc in range(4):
            t = const_pool.tile([128, W], f32, tag=f"xt{fc}")
            nc.sync.dma_start(out=t[:], in_=xT[128 * fc : 128 * (fc + 1), :])
            xt_sb.append(t)
            t2 = const_pool.tile([128, CPAD], f32, tag=f"tm{fc}")
            nc.sync.dma_start(out=t2[:], in_=Tm[128 * fc : 128 * (fc + 1), :])
            t_sb.append(t2)
            t3 = const_pool.tile([128, ROWS], f32, tag=f"xtl{fc}")
            nc.sync.dma_start(out=t3[:], in_=xTloc[128 * fc : 128 * (fc + 1), :])
            xtl_sb.append(t3)
        oh_sb = const_pool.tile([128, 32 * 128], f16, tag="onehot")
        nc.sync.dma_start(out=oh_sb[:], in_=onehot[:, :])

        ones_sb = const_pool.tile([128, 4], f16, tag="ones")
        nc.vector.memset(ones_sb[:, :], 1.0)

        # PE may carry at most one sync wait per fused matmul (walrus
        # S3_LW limit): one dummy matmul per DMA-queue sem PE will need.
        ps_dummy = build_psum.tile([128, 512], f32, tag="bld", name="ps_dummy")
        for dt_tile in (xt_sb[0], xt_sb[1], xt_sb[2], xt_sb[3], oh_sb):
            nc.tensor.matmul(
                out=ps_dummy[:, 0:512],
                lhsT=dt_tile[0:32, 0:128],
                rhs=dt_tile[0:32, 0:512],
                start=True,
                stop=True,
                tile_position=(0, 0),
            )

        # ---- build M^T (permuted slots; [256, W] as 2 tiles) -------------
        mt_sb = [
            const_pool.tile([128, W], f16, tag="mt0", name="mt0"),
            const_pool.tile([128, W], f16, tag="mt1", name="mt1"),
        ]
        for blk in range(2):
            ps = build_psum.tile([128, 512], f32, tag="bld")
            for fc in range(4):
                nc.tensor.matmul(
                    out=ps[:, :],
                    lhsT=t_sb[fc][:, 128 * blk : 128 * (blk + 1)],
                    rhs=xt_sb[fc][:, :],
                    start=(fc == 0),
                    stop=(fc == 3),
                )
            nc.scalar.copy(mt_sb[blk][:, :], ps[:, :])

        # ---- build M_local [128, 256] (same slot permutation) ------------
        mloc = const_pool.tile([128, CPAD], f32, tag="mloc")
        ps = build_psum.tile([128, 512], f32, tag="bld")
        for fc in range(4):
            nc.tensor.matmul(
                out=ps[:, :CPAD],
                lhsT=xtl_sb[fc][:],
                rhs=t_sb[fc][:],
                start=(fc == 0),
                stop=(fc == 3),
            )
        nc.scalar.copy(mloc[:], ps[:, :CPAD])

        build_ctx.close()  # release build psum banks before the main loop

        cs_pool = ctx.enter_context(tc.tile_pool(name="cspsum", bufs=1, space="PSUM"))
        bc_psum = ctx.enter_context(tc.tile_pool(name="bcpsum", bufs=4, space="PSUM"))

        feat_sb = const_pool.tile([128, K], f32, tag="feat")
        cs_ps = cs_pool.tile([128, NCHUNK * K], f32, tag="cs", name="cs_ps")

        # ---- main loop over the 50 kernels -------------------------------
        ex_tiles = {}
        for k in range(K):
            a0 = a2 = l1 = l2 = l3 = None
            for d in range(D):
                c = 5 * k + d
                s = SLOT_OF[c]
                blk, r = divmod(s, 128)
                bbase = (r // 32) * 32
                c0 = r % 32
                psd = bc_psum.tile([128, W], f32, tag="bc")
                nc.tensor.matmul(
                    out=psd[:, :],
                    lhsT=oh_sb[bbase : bbase + 32, 128 * c0 : 128 * (c0 + 1)],
                    rhs=mt_sb[blk][bbase : bbase + 32, :],
                    start=True,
                    stop=True,
                    tile_position=(bbase, 0),
                )

                if d == 0:
                    a0 = a_pool.tile([128, W], f16, tag="a0")
                    nc.scalar.activation(
                        a0[:],
                        psd[:],
                        mybir.ActivationFunctionType.Abs,
                        bias=mloc[:, s : s + 1],
                        scale=-1.0,
                    )
                elif d == 1:
                    l1 = l_pool.tile([128, W], f16, tag="l")
                    nc.vector._custom_dve(
                        ABS_DIFF_ACC,
                        out=l1[:],
                        in0=psd[:],
                        in1=a0[:],
                        s0=mloc[:, s : s + 1],
                    )
                elif d == 2:
                    a2 = a_pool.tile([128, W], f16, tag="a2")
                    nc.scalar.activation(
                        a2[:],
                        psd[:],
                        mybir.ActivationFunctionType.Abs,
                        bias=mloc[:, s : s + 1],
                        scale=-1.0,
                    )
                elif d == 3:
                    l2 = l_pool.tile([128, W], f16, tag="l")
                    nc.vector._custom_dve(
                        ABS_DIFF_ACC,
                        out=l2[:],
                        in0=psd[:],
                        in1=l1[:],
                        s0=mloc[:, s : s + 1],
                    )
                else:
                    l3 = l_pool.tile([128, W], f16, tag="l")
                    nc.vector._custom_dve(
                        ABS_DIFF_ACC,
                        out=l3[:],
                        in0=psd[:],
                        in1=l2[:],
                        s0=mloc[:, s : s + 1],
                    )

            lall = g_pool.tile([128, W], f16, tag="g")
            nc.gpsimd.tensor_tensor(
                out=lall[:], in0=l3[:], in1=a2[:], op=mybir.AluOpType.add
            )

            ex = scratch_pool.tile([128, W], f16, tag="ex")
            nc.scalar.activation(
                ex[:],
                lall[:],
                mybir.ActivationFunctionType.Exp,
                bias=0.0,
                scale=-1.0,
                accum_out=feat_sb[:, k : k + 1],
            )
            ex_tiles[k] = ex

            # column sums of the 4 off-diagonal chunks: cs[p, 50*ch+k] =
            # sum_i ex[i, 128*(ch+1)+p]  (exp tile as stationary operand).
            # Deferred 2 iterations so these PE ops (which wait on exp) sit
            # behind independent plane matmuls in the PE's in-order queue.
            for kc in ([k - 2] if k >= 2 else []) + ([K - 2, K - 1] if k == K - 1 else []):
                exc = ex_tiles.pop(kc)
                for ch in range(NCHUNK):
                    nc.tensor.matmul(
                        out=cs_ps[:, K * ch + kc : K * ch + kc + 1],
                        lhsT=exc[:, 128 * (ch + 1) : 128 * (ch + 2)],
                        rhs=ones_sb[:, 0:1],
                        start=True,
                        stop=True,
                    )

        cs_sb = const_pool.tile([128, NCHUNK * K], f32, tag="cssb")
        nc.scalar.copy(cs_sb[:], cs_ps[:])
        nc.sync.dma_start(out=feat[:, :], in_=feat_sb[:, :K])
        nc.sync.dma_start(out=csum[:, :], in_=cs_sb[:, :])

    nc.compile()
    return nc


_program_cache = {}


def _get_program():
    if "nc" not in _program_cache:
        _program_cache["nc"] = _build_program()
    return _program_cache["nc"]


def _make_onehot():
    oh = np.zeros((128, 32 * 128), dtype=np.float16)
    for p in range(128):
        oh[p, (p % 32) * 128 : (p % 32 + 1) * 128] = 1.0
    return oh


def kernel(x: np.ndarray, T: np.ndarray, _trace=False, _trace_kwargs=None):
    x = np.asarray(x, dtype=np.float32)
    T = np.asarray(T, dtype=np.float32)
    nc = _get_program()

    xT_full = np.ascontiguousarray(x.T)  # [512, 1024]
    Tm_perm = np.zeros((F, CPAD), dtype=np.float32)
    Tm_perm[:, SLOT_OF] = T
    oh = _make_onehot()
    in_maps = []
    for i in range(NCORES):
        xrot = np.roll(xT_full, -ROWS * i, axis=1)
        in_maps.append(
            {
                "xT": np.ascontiguousarray(xrot[:, :W]),
                "xTloc": np.ascontiguousarray(xrot[:, :ROWS]),
                "Tm": Tm_perm,
                "onehot": oh,
            }
        )

    res = run_bass_kernel_spmd(
        nc,
        in_maps,
        core_ids=list(range(NCORES)),
        trace=_trace,
        **(_trace_kwargs or {}),
    )
    # row sums for own rows
    feats = np.concatenate(
        [res.results[i]["feat"] for i in range(NCORES)], axis=0
    ).astype(np.float32)
    # column-sum contributions: core c's chunk ch covers rows of core
    # (c+1+ch) mod 8
    for c in range(NCORES):
        cs = res.results[c]["csum"].astype(np.float32)  # [128, 4*K]
        for ch in range(NCHUNK):
            tgt = (c + 1 + ch) % NCORES
            feats[ROWS * tgt : ROWS * (tgt + 1), :] += cs[:, K * ch : K * (ch + 1)]
    # distance-4 block pairs (absent from all device bands) on host
    M = (x @ T).reshape(B, K, D)
    for a in range(4):
        Xa = M[128 * a : 128 * (a + 1)]
        Xb = M[128 * (a + 4) : 128 * (a + 5)]
        Dif = np.abs(Xa[:, None, :, :] - Xb[None, :, :, :]).sum(-1)
        G = np.exp(-Dif)
        feats[128 * a : 128 * (a + 1)] += G.sum(1)
        feats[128 * (a + 4) : 128 * (a + 5)] += G.sum(0)
    out = np.concatenate([x, feats], axis=1)
    if _trace:
        return out, res
    return out


# revision 14
# speedup vs baseline: 1.4993x; 1.4993x over previous
"""Minibatch discrimination kernel for 8 trn2 NeuronCores.

reference:
    M = (x @ T).reshape(B, K, D)                       # B=1024, K=50, D=5
    abs_diffs[i,k,j] = sum_d |M[i,k,d] - M[j,k,d]|
    feat[i,k] = sum_j exp(-abs_diffs[i,k,j])
    out = concat([x, feat], axis=1)                    # [1024, 562]

Sharding (symmetric-banded): G[i,j,k] = exp(-abs_diffs) is symmetric in
(i,j).  Core c owns query rows [128c, 128c+128) and computes only the
key band [128c, 128c+512) mod B (its diagonal block + the next 3
128-blocks) -> unordered block pairs at distance 1..3 covered exactly
once; the 4 distance-4 pairs are added by a host-side numpy correction
(1/16 of the pair work, reusing the M = x @ T the host computes anyway).
Each band contributes row sums (Exp accum_out) for its own rows and PE
column sums (exp tile as stationary operand) for rows of cores c+1..c+3,
combined host-side.

Device inner loop, processed as 25 packed pairs of kernels (k, k+1)
with [128, 1024] tiles (columns 0:512 = k, 512:1024 = k+1):
 - The per-plane matmul produces M_j[c] - M_i[c] DIRECTLY in PSUM: the
   stationary operand is a 32-row slice of a host-built selector tile
   holding a one-hot row (selects M^T row c from the moving band tile)
   plus -M_local[:, c] in row 31, which multiplies a constant-ones row
   31 of the band tile.  No bias needed anywhere downstream, so every
   elementwise pass packs 2 kernels wide (the ~350ns per-op bubble
   dominates at width 512).  Slot groups rotate across the PE's 32-row
   quadrants.
 - Planes d=0,2: ScalarE Abs (packed).
 - Planes d=1,3,4: custom DVE op  L = |psum| + L_prev  (ABS_ACC0).
 - GpSimd adds the second scalar plane into the chain (packed).
 - ScalarE Exp(-L) per k-half with accum_out -> row sums, emitted one
   pair late so Scalar's in-order queue never head-of-line blocks.
"""

import sys

sys.path.insert(0, "/opt/trn_rl_repo")

from contextlib import ExitStack

import numpy as np

import concourse.bacc as bacc
import concourse.tile as tile
from concourse import mybir
from concourse.bass_utils import run_bass_kernel_spmd

B, F = 1024, 512
K, D = 50, 5
C = K * D  # 250 planes
NCORES = 8
ROWS = B // NCORES  # 128 query rows per core
W = 512  # key band width per core (diag block + 3 neighbours)
NCHUNK = 3  # off-diagonal 128-chunks per band
NBLK = 3  # M^T band tiles (3 x 4 groups x 31 slots = 372 >= 250)
NB = 63  # selector col-blocks (per-group counter)

f32 = mybir.dt.float32
f16 = mybir.dt.float16


# ---- custom DVE op: out = |in0| + in1 -----------------------------------
def _ensure_absacc0():
    import concourse.dve_ops as dve_ops
    from concourse.dve_spec import Spec, Src0, Src1, Zero, maxx

    for op in dve_ops.OPS:
        if op.name == "ABS_ACC0":
            return op

    def _ref(in0, in1, s0, s1, imm2):
        return (np.abs(in0.astype(np.float32)) + in1).astype(np.float32)

    op = dve_ops.DveOp(
        "ABS_ACC0",
        Spec(body=maxx(Src0, Zero - Src0) + Src1, reference=_ref),
        subdim=False,
        uops_sha={"v3": "453a5ea4d2a5cb7f", "v4": "c9b21de05de5654d"},
    )
    dve_ops.OPS.append(op)  # in place: bass_utils holds a from-import binding
    dve_ops._SUB_OPCODE_FOR_NAME[op.name] = (
        dve_ops._CUSTOM_DVE_ROW_BASE + len(dve_ops.OPS) - 1
    )
    return op


ABS_ACC0 = _ensure_absacc0()


def _slot_assignment():
    """plane c -> (group q, band tile blk, in-group row rr, col-block n).

    Groups rotate so the plane emission order (k,0),(k+1,0),(k,1),...
    of each packed pair cycles all 4 PE quadrants; the +2*(k//2) shift
    balances per-group counts to 63/63/62/62 (<= 3*31 = 93)."""
    slot = [None] * C
    nxt = {q: 0 for q in range(4)}
    for k in range(K):
        for d in range(D):
            c = 5 * k + d
            q = (2 * d + (k % 2) + 2 * (k // 2)) % 4
            n = nxt[q]
            nxt[q] += 1
            blk, rr = divmod(n, 31)
            slot[c] = (q, blk, rr, n)
    return slot


SLOT = _slot_assignment()


def _build_program():
    nc = bacc.Bacc("TRN2", target_bir_lowering=False)

    # host-built inputs (the host computes M = x @ T anyway for the
    # distance-4 correction; M^T band + selector tiles ship ready-made)
    mtb = nc.dram_tensor("mtb", [NBLK * 128, W], f16, kind="ExternalInput").ap()
    ohm = nc.dram_tensor("ohm", [128, NB * 128], f16, kind="ExternalInput").ap()
    feat = nc.dram_tensor("feat", [ROWS, K], f32, kind="ExternalOutput").ap()
    csum = nc.dram_tensor("csum", [128, NCHUNK * K], f32, kind="ExternalOutput").ap()

    with tile.TileContext(nc) as tc, ExitStack() as ctx:
        const_pool = ctx.enter_context(tc.tile_pool(name="const", bufs=1))
        cs_pool = ctx.enter_context(tc.tile_pool(name="cspsum", bufs=1, space="PSUM"))
        bc_psum = ctx.enter_context(tc.tile_pool(name="bcpsum", bufs=3, space="PSUM"))
        a_pool = ctx.enter_context(tc.tile_pool(name="apool", bufs=4))
        l_pool = ctx.enter_context(tc.tile_pool(name="lpool", bufs=6))
        g_pool = ctx.enter_context(tc.tile_pool(name="gpool", bufs=3))
        scratch_pool = ctx.enter_context(tc.tile_pool(name="scratch", bufs=10))

        # ---- load inputs -------------------------------------------------
        mt_sb = []
        for blk in range(NBLK):
            t = const_pool.tile([128, W], f16, tag=f"mt{blk}")
            nc.sync.dma_start(out=t[:], in_=mtb[128 * blk : 128 * (blk + 1), :])
            mt_sb.append(t)
        ohm_sb = const_pool.tile([128, NB * 128], f16, tag="ohm")
        nc.sync.dma_start(out=ohm_sb[:], in_=ohm[:, :])

        ones_sb = const_pool.tile([128, 4], f16, tag="ones")
        nc.vector.memset(ones_sb[:, :], 1.0)

        # PE may carry at most one sync wait per fused matmul (walrus
        # S3_LW limit): one dummy matmul per DMA-queue sem PE will need.
        ps_dummy = bc_psum.tile([128, 1024], f32, tag="bc", name="ps_dummy")
        for dt_tile in (mt_sb[0], mt_sb[1], mt_sb[2], ohm_sb):
            nc.tensor.matmul(
                out=ps_dummy[:, 0:512],
                lhsT=dt_tile[0:32, 0:128],
                rhs=dt_tile[0:32, 0:512],
                start=True,
                stop=True,
                tile_position=(0, 0),
            )

        feat_sb = const_pool.tile([128, K], f32, tag="feat")
        cs_ps = cs_pool.tile([128, NCHUNK * K], f32, tag="cs", name="cs_ps")

        # ---- main loop: 25 packed pairs of kernels -----------------------
        ex_tiles = {}
        exp_pending = []
        for k in range(0, K, 2):
            a0 = a2 = l1 = l2 = l3 = None
            for d in range(D):
                psd = bc_psum.tile([128, 1024], f32, tag="bc")
                for half in range(2):
                    c = 5 * (k + half) + d
                    q, blk, rr, n = SLOT[c]
                    nc.tensor.matmul(
                        out=psd[:, 512 * half : 512 * (half + 1)],
                        lhsT=ohm_sb[32 * q : 32 * q + 32, 128 * n : 128 * (n + 1)],
                        rhs=mt_sb[blk][32 * q : 32 * q + 32, :],
                        start=True,
                        stop=True,
                        tile_position=(32 * q, 0),
                    )

                if d == 0:
                    a0 = a_pool.tile([128, 1024], f16, tag="a0")
                    nc.scalar.activation(
                        a0[:], psd[:], mybir.ActivationFunctionType.Abs
                    )
                elif d == 1:
                    l1 = l_pool.tile([128, 1024], f16, tag="l")
                    nc.vector._custom_dve(ABS_ACC0, out=l1[:], in0=psd[:], in1=a0[:])
                elif d == 2:
                    a2 = a_pool.tile([128, 1024], f16, tag="a2")
                    nc.scalar.activation(
                        a2[:], psd[:], mybir.ActivationFunctionType.Abs
                    )
                elif d == 3:
                    l2 = l_pool.tile([128, 1024], f16, tag="l")
                    nc.vector._custom_dve(ABS_ACC0, out=l2[:], in0=psd[:], in1=l1[:])
                else:
                    l3 = l_pool.tile([128, 1024], f16, tag="l")
                    nc.vector._custom_dve(ABS_ACC0, out=l3[:], in0=psd[:], in1=l2[:])

            lall = g_pool.tile([128, 1024], f16, tag="g")
            nc.gpsimd.tensor_tensor(
                out=lall[:], in0=l3[:], in1=a2[:], op=mybir.AluOpType.add
            )

            # exps emitted one pair late: Scalar's in-order queue would
            # otherwise head-of-line block on lall (gpsimd) while the next
            # pair's Abs inputs already sit in PSUM
            exp_pending.append((k, lall))
            todo = [exp_pending.pop(0)] if len(exp_pending) > 1 else []
            if k == K - 2:
                todo += exp_pending
                exp_pending = []
            for kk0, lt in todo:
                for half in range(2):
                    kk = kk0 + half
                    ex = scratch_pool.tile([128, W], f16, tag="ex")
                    nc.scalar.activation(
                        ex[:],
                        lt[:, 512 * half : 512 * (half + 1)],
                        mybir.ActivationFunctionType.Exp,
                        bias=0.0,
                        scale=-1.0,
                        accum_out=feat_sb[:, kk : kk + 1],
                    )
                    ex_tiles[kk] = ex

            # column sums (exp tile stationary), deferred 2 pairs so these
            # PE ops sit behind independent plane matmuls in the PE queue
            done = [k - 4, k - 3] if k >= 4 else []
            if k == K - 2:
                done += [K - 4, K - 3, K - 2, K - 1]
            for kc in done:
                exc = ex_tiles.pop(kc)
                for ch in range(NCHUNK):
                    nc.tensor.matmul(
                        out=cs_ps[:, K * ch + kc : K * ch + kc + 1],
                        lhsT=exc[:, 128 * (ch + 1) : 128 * (ch + 2)],
                        rhs=ones_sb[:, 0:1],
                        start=True,
                        stop=True,
                    )

        cs_sb = const_pool.tile([128, NCHUNK * K], f32, tag="cssb")
        nc.scalar.copy(cs_sb[:], cs_ps[:])
        nc.sync.dma_start(out=feat[:, :], in_=feat_sb[:, :K])
        nc.sync.dma_start(out=csum[:, :], in_=cs_sb[:, :])

    nc.compile()
    return nc


_program_cache = {}


def _get_program():
    if "nc" not in _program_cache:
        _program_cache["nc"] = _build_program()
    return _program_cache["nc"]


def _ohm_base():
    """Constant one-hot part of the selector tile (bias rows filled per
    core at call time)."""
    oh = np.zeros((128, NB * 128), dtype=np.float16)
    for c in range(C):
        q, blk, rr, n = SLOT[c]
        oh[32 * q + rr, 128 * n : 128 * (n + 1)] = 1.0
    return oh


_OHM_BASE = _ohm_base()


def kernel(x: np.ndarray, T: np.ndarray, _trace=False, _trace_kwargs=None):
    x = np.asarray(x, dtype=np.float32)
    T = np.asarray(T, dtype=np.float32)
    nc = _get_program()

    M2 = x @ T  # [B, C] -- also reused for the distance-4 correction
    M2_16 = M2.astype(np.float16)

    qv = np.array([s[0] for s in SLOT])
    blkv = np.array([s[1] for s in SLOT])
    rrv = np.array([s[2] for s in SLOT])
    nv = np.array([s[3] for s in SLOT])
    slot_row = 128 * blkv + 32 * qv + rrv  # row in mtb per plane

    in_maps = []
    for i in range(NCORES):
        band = (np.arange(W) + ROWS * i) % B
        mtb = np.zeros((NBLK * 128, W), dtype=np.float16)
        mtb[slot_row, :] = M2_16[band, :].T  # [C, W]
        for blk in range(NBLK):
            for q in range(4):
                mtb[128 * blk + 32 * q + 31, :] = 1.0  # constant-ones rows
        ohm = _OHM_BASE.copy()
        Mloc = M2_16[ROWS * i : ROWS * (i + 1), :]  # [128, C]
        # bias rows: ohm[32q+31, 128n + i'] = -Mloc[i', c]
        bias = np.zeros((4, NB * 128), dtype=np.float16)
        bias[qv[:, None], (128 * nv)[:, None] + np.arange(128)[None, :]] = -Mloc.T
        for q in range(4):
            ohm[32 * q + 31, :] = bias[q]
        in_maps.append({"mtb": mtb, "ohm": ohm})

    res = run_bass_kernel_spmd(
        nc,
        in_maps,
        core_ids=list(range(NCORES)),
        trace=_trace,
        **(_trace_kwargs or {}),
    )
    # row sums for own rows
    feats = np.concatenate(
        [res.results[i]["feat"] for i in range(NCORES)], axis=0
    ).astype(np.float32)
    # column-sum contributions: core c's chunk ch covers rows of core
    # (c+1+ch) mod 8
    for c in range(NCORES):
        cs = res.results[c]["csum"].astype(np.float32)  # [128, 3*K]
        for ch in range(NCHUNK):
            tgt = (c + 1 + ch) % NCORES
            feats[ROWS * tgt : ROWS * (tgt + 1), :] += cs[:, K * ch : K * (ch + 1)]
    # distance-4 block pairs (absent from all device bands) on host
    M = M2.reshape(B, K, D)
    for a in range(4):
        Xa = M[128 * a : 128 * (a + 1)]
        Xb = M[128 * (a + 4) : 128 * (a + 5)]
        Dif = np.abs(Xa[:, None, :, :] - Xb[None, :, :, :]).sum(-1)
        G = np.exp(-Dif)
        feats[128 * a : 128 * (a + 1)] += G.sum(1)
        feats[128 * (a + 4) : 128 * (a + 5)] += G.sum(0)
    out = np.concatenate([x, feats], axis=1)
    if _trace:
        return out, res
    return out
